# revision 4
# baseline (speedup 1.0000x reference)
"""Trainium2 Bass kernel for nn_Middle_Moudle_v3 (retrieval_knn).

For each episode (b, s): cosine similarity of every support spatial C-vector
against every query spatial C-vector, max over query positions.

  support_x, query_x: [8, 75, 64, 19, 19] fp32  ->  out [8, 75, 361] fp32

Sharding: data-parallel over the leading batch dim (8 episodes -> 8 cores).

Per-core plan (75 (b,s) pairs, padded to 76 = 38 two-pair tiles of [128, 361]):
  - load support/query tiles [128, 361] (partitions = (pair, channel))
  - ACT squares + PE "shifted-window ones" matmuls accumulate per-group
    sumsq banks [16, 361] in PSUM
  - ACT Ln/Exp(-0.5) -> reciprocal norms; query norms bounce through DRAM and
    reload partition-broadcast as [128, 361] tiles (rq2)
  - DVE pre-scales query (qh = q * rq2); PE computes the 361x361 cosine GEMM
    in 3 output chunks per pair (K=64, two pairs pack the array rows);
    DVE does one batched 3-bank max-reduce per pair
  - support norms are applied post-max: PE-transpose rs rows, DVE multiply,
    store output transposed as [361, 76] (host transposes back)

A post-pass splits multi-wait instructions (this walrus build enforces the
one-events-slot-per-instruction ISA limit instead of splitting itself).
"""
import numpy as np

import concourse.bass as bass
import concourse.mybir as mybir
import concourse.tile as tile
from concourse.bass_utils import run_bass_kernel_spmd

F32 = mybir.dt.float32
B = 8          # episodes = cores
S = 75         # (b, s) pairs per core
SP = 76        # padded pairs
NT = SP // 2   # 38 two-pair tiles
C = 64         # channels
N = 361        # spatial positions (19*19)
GROUPS = [(0, 16), (16, 16), (32, 6)]
CHUNKS = [(0, 128), (128, 128), (256, 105)]  # (offset, mc) output chunks

_ws_ctr = [0]


def _split_multi_waits(nc):
    """Move all-but-one sync wait of each instruction onto injected
    InstEventSemaphore instructions (standalone sequencer waits)."""
    for f in nc.m.functions:
        for bb in f.blocks:
            insts = list(bb.instructions)
            out = []
            changed = False
            for ins in insts:
                si = ins.sync_info
                if si is not None and len(si.on_wait) > 1:
                    waits = list(si.on_wait)
                    for w in waits[:-1]:
                        _ws_ctr[0] += 1
                        ev = mybir.InstEventSemaphore(
                            name=f"wsplit_{_ws_ctr[0]}",
                            engine=ins.engine,
                            sync_info=mybir.SyncInfo(on_wait=[w], on_update=[]),
                        )
                        out.append(ev)
                    ins.sync_info = mybir.SyncInfo(
                        on_wait=[waits[-1]], on_update=list(si.on_update)
                    )
                    changed = True
                out.append(ins)
            if changed:
                bb.instructions = out


def _build_nc():
    # constants baked into the NEFF
    win_np = np.zeros((128, 62), dtype=np.float32)
    win_np[0:C, 30] = 1.0
    win_np[C:128, 31] = 1.0
    ident_np = np.eye(128, dtype=np.float32)

    nc = bass.Bass(target_bir_lowering=False)
    sup_d = nc.dram_tensor("support", [SP * C, N], F32, kind="ExternalInput")
    qry_d = nc.dram_tensor("query", [SP * C, N], F32, kind="ExternalInput")
    out_d = nc.dram_tensor("out", [N, SP], F32, kind="ExternalOutput")
    rq_scr_d = nc.dram_tensor("rq_scr", [SP, N], F32)
    win_d = nc.inline_tensor(win_np, name="win")
    ident_d = nc.inline_tensor(ident_np, name="ident")

    with tile.TileContext(nc) as tc:
        with tc.tile_pool(name="inp", bufs=NT) as inp, \
             tc.tile_pool(name="work", bufs=1) as work, \
             tc.tile_pool(name="sqp", bufs=3) as sqp, \
             tc.tile_pool(name="qhp", bufs=10) as qhp, \
             tc.tile_pool(name="rq2p", bufs=6) as rq2p, \
             tc.tile_pool(name="tmpp", bufs=2) as tmpp, \
             tc.tile_pool(name="psn", bufs=1, space="PSUM") as psn, \
             tc.tile_pool(name="psd", bufs=2, space="PSUM") as psd:

            win_sb = work.tile([128, 62], F32)
            ident_sb = work.tile([128, 128], F32)
            nc.sync.dma_start(win_sb[:], win_d[:])
            nc.sync.dma_start(ident_sb[:], ident_d[:])

            rq_rows = work.tile([SP, N], F32)
            rs_rows = work.tile([SP, N], F32)
            colmax = work.tile([128, 3 * SP], F32)  # col 3*P+m

            qt = [None] * NT
            st = [None] * NT
            for j in range(NT):
                qt[j] = inp.tile([128, N], F32, tag="qt", name=f"qt{j}")
                st[j] = inp.tile([128, N], F32, tag="st", name=f"st{j}")
                nc.sync.dma_start(qt[j][:], qry_d[128 * j:128 * j + 128, :])
                nc.sync.dma_start(st[j][:], sup_d[128 * j:128 * j + 128, :])

            for j0, T in GROUPS:
                bank_q = psn.tile([32, 512], F32, tag="bank_q")
                bank_s = psn.tile([32, 512], F32, tag="bank_s")
                # squares + windowed ones-matmul accumulation of sumsq rows
                for l in range(T):
                    j = j0 + l
                    lhsT = win_sb[:, 30 - 2 * l:62 - 2 * l]
                    sqs = sqp.tile([128, N], F32, tag="sq")
                    nc.scalar.square(sqs[:], st[j][:])
                    nc.tensor.matmul(bank_s[0:32, 0:N], lhsT, sqs[:],
                                     start=(l == 0), stop=(l == T - 1))
                    sqq = sqp.tile([128, N], F32, tag="sq")
                    nc.scalar.square(sqq[:], qt[j][:])
                    nc.tensor.matmul(bank_q[0:32, 0:N], lhsT, sqq[:],
                                     start=(l == 0), stop=(l == T - 1))
                # reciprocal norms: exp(-0.5 * ln(sumsq))
                r0 = 2 * j0
                nr = 2 * T
                tmq = tmpp.tile([32, N], F32, tag="tmq")
                nc.scalar.activation(tmq[0:nr, :], bank_q[0:nr, 0:N],
                                     mybir.ActivationFunctionType.Ln)
                nc.scalar.activation(rq_rows[r0:r0 + nr, :], tmq[0:nr, :],
                                     mybir.ActivationFunctionType.Exp, scale=-0.5)
                tms = tmpp.tile([32, N], F32, tag="tms")
                nc.scalar.activation(tms[0:nr, :], bank_s[0:nr, 0:N],
                                     mybir.ActivationFunctionType.Ln)
                nc.scalar.activation(rs_rows[r0:r0 + nr, :], tms[0:nr, :],
                                     mybir.ActivationFunctionType.Exp, scale=-0.5)
                # bounce rq rows via DRAM (scalar-engine DMA queue)
                nc.scalar.dma_start(rq_scr_d[r0:r0 + nr, :], rq_rows[r0:r0 + nr, :])

                # main GEMM + fused reduce for this group
                for l in range(T):
                    j = j0 + l
                    rq2 = rq2p.tile([128, N], F32, tag="rq2")
                    for e in range(2):
                        row = rq_scr_d[2 * j + e:2 * j + e + 1, :]
                        bc = bass.AP(tensor=row.tensor, offset=row.offset,
                                     ap=[[0, C], [1, N]])
                        nc.scalar.dma_start(rq2[C * e:C * e + C, :], bc)
                    qh = qhp.tile([128, N], F32, tag="qh")
                    nc.vector.tensor_tensor(out=qh[:], in0=qt[j][:], in1=rq2[:],
                                            op=mybir.AluOpType.mult)
                    for e in range(2):
                        P = 2 * j + e
                        dot = psd.tile([128, 3, 512], F32, tag="dot")
                        for m, (off, mc) in enumerate(CHUNKS):
                            nc.tensor.matmul(
                                dot[0:mc, m, 0:N],
                                st[j][C * e:C * e + C, off:off + mc],
                                qh[C * e:C * e + C, 0:N],
                                start=True, stop=True,
                            )
                        nc.vector.tensor_reduce(
                            colmax[:, 3 * P:3 * P + 3], dot[:, :, 0:N],
                            axis=mybir.AxisListType.X, op=mybir.AluOpType.max,
                        )

            # tail: transpose rs rows, apply, store transposed output
            for m, (off, mc) in enumerate(CHUNKS):
                tp = psn.tile([128, 512], F32, tag="bank_q")
                nc.tensor.transpose(tp[0:mc, 0:SP], rs_rows[:, off:off + mc],
                                    ident_sb[0:SP, 0:SP])
                rs_t = work.tile([128, SP], F32, tag=f"rs_t{m}")
                nc.vector.tensor_copy(rs_t[0:mc, :], tp[0:mc, 0:SP])
                fin = work.tile([128, SP], F32, tag=f"fin{m}")
                nc.vector.tensor_tensor(out=fin[0:mc, :], in0=colmax[0:mc, m::3],
                                        in1=rs_t[0:mc, :], op=mybir.AluOpType.mult)
                nc.sync.dma_start(out_d[off:off + mc, :], fin[0:mc, :])

    _split_multi_waits(nc)
    return nc


_NC_CACHE = None


def _get_nc():
    global _NC_CACHE
    if _NC_CACHE is None:
        _NC_CACHE = _build_nc()
    return _NC_CACHE


def kernel(support_x, query_x, **_unused):
    sup = np.asarray(support_x, dtype=np.float32).reshape(B, S, C, N)
    qry = np.asarray(query_x, dtype=np.float32).reshape(B, S, C, N)
    # pad pair 75 with a copy of pair 74
    sup_p = np.concatenate([sup, sup[:, S - 1:S]], axis=1).reshape(B, SP * C, N)
    qry_p = np.concatenate([qry, qry[:, S - 1:S]], axis=1).reshape(B, SP * C, N)
    sup_p = np.ascontiguousarray(sup_p)
    qry_p = np.ascontiguousarray(qry_p)

    nc = _get_nc()
    in_maps = [{"support": sup_p[b], "query": qry_p[b]} for b in range(B)]
    res = run_bass_kernel_spmd(nc, in_maps, core_ids=list(range(B)))
    out = np.stack([res.results[b]["out"].T[:S] for b in range(B)])
    return np.ascontiguousarray(out, dtype=np.float32)


# revision 6
# speedup vs baseline: 1.1713x; 1.1713x over previous
"""Trainium2 Bass kernel for nn_Middle_Moudle_v3 (retrieval_knn).

For each episode (b, s): cosine similarity of every support spatial C-vector
against every query spatial C-vector, max over query positions.

  support_x, query_x: [8, 75, 64, 19, 19] fp32  ->  out [8, 75, 361] fp32

Sharding: data-parallel over the leading batch dim (8 episodes -> 8 cores).

Per-core plan (75 (b,s) pairs, padded to 76 = 38 two-pair tiles of [128, 361]):
  - load support/query tiles [128, 361] (partitions = (pair, channel))
  - ACT squares + PE "shifted-window ones" matmuls accumulate per-group
    sumsq banks [16, 361] in PSUM
  - ACT Ln/Exp(-0.5) -> reciprocal norms; query norms bounce through DRAM and
    reload partition-broadcast as [128, 361] tiles (rq2)
  - DVE pre-scales query (qh = q * rq2); PE computes the 361x361 cosine GEMM
    in 3 output chunks per pair (K=64, two pairs pack the array rows);
    DVE does one batched 3-bank max-reduce per pair
  - support norms are applied post-max: PE-transpose rs rows, DVE multiply,
    store output transposed as [361, 76] (host transposes back)

A post-pass splits multi-wait instructions (this walrus build enforces the
one-events-slot-per-instruction ISA limit instead of splitting itself).
"""
import numpy as np

import concourse.bass as bass
import concourse.mybir as mybir
import concourse.tile as tile
from concourse.bass_utils import run_bass_kernel_spmd

F32 = mybir.dt.float32
B = 8          # episodes = cores
S = 75         # (b, s) pairs per core
SP = 76        # padded pairs
NT = SP // 2   # 38 two-pair tiles
C = 64         # channels
N = 361        # spatial positions (19*19)
GROUPS = [(0, 16), (16, 16), (32, 6)]
CHUNKS = [(0, 128), (128, 128), (256, 105)]  # (offset, mc) output chunks

_ws_ctr = [0]


def _split_multi_waits(nc):
    """Move all-but-one sync wait of each instruction onto injected
    InstEventSemaphore instructions (standalone sequencer waits)."""
    for f in nc.m.functions:
        for bb in f.blocks:
            insts = list(bb.instructions)
            out = []
            changed = False
            for ins in insts:
                si = ins.sync_info
                if si is not None and len(si.on_wait) > 1:
                    waits = list(si.on_wait)
                    for w in waits[:-1]:
                        _ws_ctr[0] += 1
                        ev = mybir.InstEventSemaphore(
                            name=f"wsplit_{_ws_ctr[0]}",
                            engine=ins.engine,
                            sync_info=mybir.SyncInfo(on_wait=[w], on_update=[]),
                        )
                        out.append(ev)
                    ins.sync_info = mybir.SyncInfo(
                        on_wait=[waits[-1]], on_update=list(si.on_update)
                    )
                    changed = True
                out.append(ins)
            if changed:
                bb.instructions = out


def _build_nc(repeats=None):
    # constants baked into the NEFF
    win_np = np.zeros((128, 62), dtype=np.float32)
    win_np[0:C, 30] = 1.0
    win_np[C:128, 31] = 1.0
    ident_np = np.eye(128, dtype=np.float32)

    nc = bass.Bass(target_bir_lowering=False)
    sup_d = nc.dram_tensor("support", [SP * C, N], F32, kind="ExternalInput")
    qry_d = nc.dram_tensor("query", [SP * C, N], F32, kind="ExternalInput")
    out_d = nc.dram_tensor("out", [N, SP], F32, kind="ExternalOutput")
    rq_scr_d = nc.dram_tensor("rq_scr", [SP, N], F32)
    win_d = nc.inline_tensor(win_np, name="win")
    ident_d = nc.inline_tensor(ident_np, name="ident")

    with tile.TileContext(nc) as tc:
        with tc.tile_pool(name="inp", bufs=NT) as inp, \
             tc.tile_pool(name="work", bufs=1) as work, \
             tc.tile_pool(name="sqp", bufs=3) as sqp, \
             tc.tile_pool(name="qhp", bufs=10) as qhp, \
             tc.tile_pool(name="rq2p", bufs=6) as rq2p, \
             tc.tile_pool(name="tmpp", bufs=2) as tmpp, \
             tc.tile_pool(name="psn", bufs=1, space="PSUM") as psn, \
             tc.tile_pool(name="psd", bufs=2, space="PSUM") as psd:

            win_sb = work.tile([128, 62], F32)
            ident_sb = work.tile([128, 128], F32)
            nc.sync.dma_start(win_sb[:], win_d[:])
            nc.sync.dma_start(ident_sb[:], ident_d[:])

            rq_rows = work.tile([SP, N], F32)
            rs_rows = work.tile([SP, N], F32)
            colmax = work.tile([128, 3 * SP], F32)  # col 3*P+m

            qt = [None] * NT
            st = [None] * NT
            for j in range(NT):
                qt[j] = inp.tile([128, N], F32, tag="qt", name=f"qt{j}")
                st[j] = inp.tile([128, N], F32, tag="st", name=f"st{j}")
                nc.sync.dma_start(qt[j][:], qry_d[128 * j:128 * j + 128, :])
                nc.sync.dma_start(st[j][:], sup_d[128 * j:128 * j + 128, :])

            def body():
                _kernel_body(nc, tc, qt, st, win_sb, ident_sb, rq_rows, rs_rows,
                             colmax, work, sqp, qhp, rq2p, tmpp, psn, psd,
                             rq_scr_d, out_d)

            if repeats is None:
                body()
            else:
                with tc.For_i(0, repeats, 1):
                    body()

    _split_multi_waits(nc)
    return nc


def _kernel_body(nc, tc, qt, st, win_sb, ident_sb, rq_rows, rs_rows, colmax,
                 work, sqp, qhp, rq2p, tmpp, psn, psd, rq_scr_d, out_d):
            for j0, T in GROUPS:
                bank_q = psn.tile([32, 512], F32, tag="bank_q")
                bank_s = psn.tile([32, 512], F32, tag="bank_s")
                # squares + windowed ones-matmul accumulation of sumsq rows
                for l in range(T):
                    j = j0 + l
                    lhsT = win_sb[:, 30 - 2 * l:62 - 2 * l]
                    sqs = sqp.tile([128, N], F32, tag="sq")
                    nc.scalar.square(sqs[:], st[j][:])
                    nc.tensor.matmul(bank_s[0:32, 0:N], lhsT, sqs[:],
                                     start=(l == 0), stop=(l == T - 1))
                    sqq = sqp.tile([128, N], F32, tag="sq")
                    nc.scalar.square(sqq[:], qt[j][:])
                    nc.tensor.matmul(bank_q[0:32, 0:N], lhsT, sqq[:],
                                     start=(l == 0), stop=(l == T - 1))
                # reciprocal norms: exp(-0.5 * ln(sumsq))
                r0 = 2 * j0
                nr = 2 * T
                tmq = tmpp.tile([32, N], F32, tag="tmq")
                nc.scalar.activation(tmq[0:nr, :], bank_q[0:nr, 0:N],
                                     mybir.ActivationFunctionType.Ln)
                nc.scalar.activation(rq_rows[r0:r0 + nr, :], tmq[0:nr, :],
                                     mybir.ActivationFunctionType.Exp, scale=-0.5)
                tms = tmpp.tile([32, N], F32, tag="tms")
                nc.scalar.activation(tms[0:nr, :], bank_s[0:nr, 0:N],
                                     mybir.ActivationFunctionType.Ln)
                nc.scalar.activation(rs_rows[r0:r0 + nr, :], tms[0:nr, :],
                                     mybir.ActivationFunctionType.Exp, scale=-0.5)
                # bounce rq rows via DRAM (scalar-engine DMA queue)
                nc.scalar.dma_start(rq_scr_d[r0:r0 + nr, :], rq_rows[r0:r0 + nr, :])

                # main GEMM + fused reduce for this group
                for l in range(T):
                    j = j0 + l
                    rq2 = rq2p.tile([128, N], F32, tag="rq2")
                    for e in range(2):
                        row = rq_scr_d[2 * j + e:2 * j + e + 1, :]
                        bc = bass.AP(tensor=row.tensor, offset=row.offset,
                                     ap=[[0, C], [1, N]])
                        nc.scalar.dma_start(rq2[C * e:C * e + C, :], bc)
                    qh = qhp.tile([128, N], F32, tag="qh")
                    nc.vector.tensor_tensor(out=qh[:], in0=qt[j][:], in1=rq2[:],
                                            op=mybir.AluOpType.mult)
                    for e in range(2):
                        P = 2 * j + e
                        dot = psd.tile([128, 3, 512], F32, tag="dot")
                        for m, (off, mc) in enumerate(CHUNKS):
                            nc.tensor.matmul(
                                dot[0:mc, m, 0:N],
                                st[j][C * e:C * e + C, off:off + mc],
                                qh[C * e:C * e + C, 0:N],
                                start=True, stop=True,
                            )
                        nc.vector.tensor_reduce(
                            colmax[:, 3 * P:3 * P + 3], dot[:, :, 0:N],
                            axis=mybir.AxisListType.X, op=mybir.AluOpType.max,
                        )

            # tail: transpose rs rows, apply, store transposed output
            for m, (off, mc) in enumerate(CHUNKS):
                tp = psn.tile([128, 512], F32, tag="bank_q")
                nc.tensor.transpose(tp[0:mc, 0:SP], rs_rows[:, off:off + mc],
                                    ident_sb[0:SP, 0:SP])
                rs_t = work.tile([128, SP], F32, tag=f"rs_t{m}")
                nc.vector.tensor_copy(rs_t[0:mc, :], tp[0:mc, 0:SP])
                fin = work.tile([128, SP], F32, tag=f"fin{m}")
                nc.vector.tensor_tensor(out=fin[0:mc, :], in0=colmax[0:mc, m::3],
                                        in1=rs_t[0:mc, :], op=mybir.AluOpType.mult)
                nc.sync.dma_start(out_d[off:off + mc, :], fin[0:mc, :])


_NC_CACHE = None


def _get_nc():
    global _NC_CACHE
    if _NC_CACHE is None:
        _NC_CACHE = _build_nc()
    return _NC_CACHE


def kernel(support_x, query_x, **_unused):
    sup = np.asarray(support_x, dtype=np.float32).reshape(B, S, C, N)
    qry = np.asarray(query_x, dtype=np.float32).reshape(B, S, C, N)
    # pad pair 75 with a copy of pair 74
    sup_p = np.concatenate([sup, sup[:, S - 1:S]], axis=1).reshape(B, SP * C, N)
    qry_p = np.concatenate([qry, qry[:, S - 1:S]], axis=1).reshape(B, SP * C, N)
    sup_p = np.ascontiguousarray(sup_p)
    qry_p = np.ascontiguousarray(qry_p)

    nc = _get_nc()
    in_maps = [{"support": sup_p[b], "query": qry_p[b]} for b in range(B)]
    res = run_bass_kernel_spmd(nc, in_maps, core_ids=list(range(B)))
    out = np.stack([res.results[b]["out"].T[:S] for b in range(B)])
    return np.ascontiguousarray(out, dtype=np.float32)
